# revision 26
# baseline (speedup 1.0000x reference)
"""DeBERTa-bare Trainium2 Bass kernel.

Topology: 8 NeuronCores = 4 data-parallel pairs (one batch element each) x
2-way tensor parallel (heads + FFN split) with pairwise AllReduce.

Everything on-chip runs feature-major ("transposed"): h is kept as
hT[d, token].  The DeBERTa disentangled-attention gathers
(take_along_axis over relative positions) are realized as affine "skew"
access-pattern DMA reads from DRAM-resident, clamp-extended c2p/p2c tables
(fp8, x256 scaled), injected into the score PSUM via scaled-identity
matmuls.

The pos_k/pos_q projections of the rel_embeddings table are precomputed on
the host (they only depend on weights).  The AllReduce/LayerNorm stream
runs in fp16, chunked over token halves so the collective overlaps the
other half's GEMMs; LN2 of each layer is deferred into the next layer's
QKV phase for the same reason.
"""

import sys

for _p in ("/opt/trn_rl_repo",):
    if _p not in sys.path:
        sys.path.insert(0, _p)

import numpy as np
import ml_dtypes

import concourse.bass as bass
import concourse.bacc as bacc
import concourse.tile as tile
import concourse.mybir as mybir
from concourse.masks import make_identity

F32 = mybir.dt.float32
BF16 = mybir.dt.bfloat16
FP16 = mybir.dt.float16
FP8 = mybir.dt.float8e4
I16 = mybir.dt.int16

AF = mybir.ActivationFunctionType
OP = mybir.AluOpType

NEG = -1e9


def mm_acc(nc, ps, lhsT3, rhs3, nsub, start, stop):
    """Accumulating matmul over `nsub` 128-contraction subtiles.
    lhsT3/rhs3: APs shaped [128, nsub, *]."""
    for s in range(nsub):
        nc.tensor.matmul(ps, lhsT3[:, s], rhs3[:, s],
                         start=(start and s == 0), stop=(stop and s == nsub - 1))


class Cfg:
    def __init__(self, B=4, S=1024, D=1024, H=16, F=4096, L=4, V=32000, SPAN=512,
                 n_cores=8, act="gelu", no_cc=False):
        self.B, self.S, self.D, self.H, self.F, self.L, self.V, self.SPAN = (
            B, S, D, H, F, L, V, SPAN)
        self.n_cores = n_cores
        self.DH = D // H
        assert self.DH == 64
        self.DT = D // 128          # d tiles
        self.TT = S // 128          # token tiles
        self.NHL = H // 2           # heads per core
        self.DCL = self.NHL * self.DH   # local head-dim cols
        self.JT = self.DCL // 128   # local dcol tiles (2 heads per tile)
        self.FL = F // 2            # local ffn cols
        self.FT = self.FL // 128
        self.CH = min(512, S)       # token chunk
        self.NCH = S // self.CH
        self.SUB = min(4, self.DT)
        self.W2 = S + 128           # compact per-tile skew table width
        self.PW = 2176              # host pos tables, clamp-extended
        self.scale = 1.0 / np.sqrt(3.0 * self.DH)
        self.act = act
        self.no_cc = no_cc
        self.ar_chunked = False     # one AR per LN point vs one per chunk
        self.ar_f32 = True          # AllReduce stream dtype (f32: CCE 16-bit path is slow)


def build_nc(cfg):
    c = cfg
    nc = bacc.Bacc("TRN2", target_bir_lowering=False, debug=False,
                   num_devices=c.n_cores)

    def inp(name, shape, dt):
        return nc.dram_tensor(name, list(shape), dt, kind="ExternalInput")

    ids16 = inp("ids16", [128, c.S // 16], I16)
    tok_emb = inp("tok_emb", [c.V, c.D], F32)
    segsel = inp("segsel", [128, c.TT], F32)
    seg0rep = inp("seg0rep", [128, c.D], F32)
    segdrep = inp("segdrep", [128, c.D], F32)
    maskt = inp("maskt", [128, c.TT], F32)
    maskbias = inp("maskbias", [128, c.TT], F32)
    egrep = inp("egrep", [128, c.D], F32)
    ebrep = inp("ebrep", [128, c.D], F32)
    poskt = inp("poskt", [c.L, 128, c.JT, c.PW], BF16)
    posqt = inp("posqt", [c.L, 128, c.JT, c.PW], BF16)
    wqkv = inp("wqkv", [c.L, 128, c.DT, 3 * c.DCL], BF16)
    bqkv = inp("bqkv", [c.L, 128, 2 * c.JT], F32)
    bvrep = inp("bvrep", [c.L, 128, c.DCL], F32)
    wo = inp("wo", [c.L, 128, c.JT, c.D], BF16)
    bo2 = inp("bo2", [c.L, 1, c.D], BF16)
    w1 = inp("w1", [c.L, 128, c.DT, c.FL], BF16)
    b1 = inp("b1", [c.L, 128, c.FT], F32)
    w2 = inp("w2", [c.L, 128, c.FT, c.D], BF16)
    b22 = inp("b22", [c.L, 1, c.D], BF16)
    ln1g = inp("ln1g", [c.L, 128, c.DT], F32)
    ln1b = inp("ln1b", [c.L, 128, c.DT], F32)
    ln2g = inp("ln2g", [c.L, 128, c.DT], F32)
    ln2b = inp("ln2b", [c.L, 128, c.DT], F32)

    out_hT = nc.dram_tensor("out_hT", [128, c.DT, c.S], F32, kind="ExternalOutput")

    pairs = [[2 * i, 2 * i + 1] for i in range(c.n_cores // 2)]

    with tile.TileContext(nc) as tc:
        import contextlib
        est = contextlib.ExitStack()
        with est:
            const = est.enter_context(tc.tile_pool(name="const", bufs=1))
            resid = est.enter_context(tc.tile_pool(name="resid", bufs=1))
            dramp = est.enter_context(tc.tile_pool(name="dramp", bufs=3, space="DRAM"))

            identT = const.tile([128, 128], F32)
            make_identity(nc, identT[:])
            ident8 = const.tile([128, 128], FP8)
            nc.gpsimd.memset(ident8[:], 2.0 ** -8)
            nc.gpsimd.affine_select(
                out=ident8[:], in_=ident8[:], compare_op=OP.is_equal, fill=0.0,
                base=0, pattern=[[-1, 128]], channel_multiplier=1)
            ones1x64 = const.tile([1, 64], F32)
            nc.vector.memset(ones1x64[:], 1.0)
            ones16 = const.tile([128, 1], BF16)
            nc.vector.memset(ones16[:], 1.0)
            ones32 = const.tile([128, 1], F32)
            nc.vector.memset(ones32[:], 1.0)
            onesrow = const.tile([1, c.CH], BF16)
            nc.vector.memset(onesrow[:], 1.0)
            eps2 = const.tile([1, 1], F32)
            nc.vector.memset(eps2[:], float(c.D) ** 2 * 1e-12)
            invD_row = const.tile([1, 128], F32)
            nc.vector.memset(invD_row[:], 1.0 / c.D)
            D_row = const.tile([1, 128], F32)
            nc.vector.memset(D_row[:], float(c.D))
            mb_sb = const.tile([128, c.TT], F32)
            nc.sync.dma_start(mb_sb[:], maskbias.ap())

            hT32 = resid.tile([128, c.DT, c.S], F32)
            hTbf = resid.tile([128, c.DT, c.S], BF16)

            consts = dict(identT=identT, ident8=ident8, ones1x64=ones1x64,
                          ones16=ones16, ones32=ones32, onesrow=onesrow,
                          eps2=eps2, invD_row=invD_row, D_row=D_row,
                          mb_sb=mb_sb)
            ins = dict(poskt=poskt, posqt=posqt, wqkv=wqkv, bqkv=bqkv,
                       bvrep=bvrep, wo=wo, bo2=bo2, w1=w1, b1=b1, w2=w2,
                       b22=b22, ln1g=ln1g, ln1b=ln1b, ln2g=ln2g, ln2b=ln2b)

            # ---------------- embedding ----------------
            with (
                tc.tile_pool(name="embp", bufs=1) as embp,
                tc.tile_pool(name="embps", bufs=2, space="PSUM") as embps,
            ):
                ids_sb = embp.tile([128, c.S // 16], I16)
                nc.sync.dma_start(ids_sb[:], ids16.ap())
                gb = embp.tile([128, c.TT, c.D], F32)
                nc.gpsimd.dma_gather(
                    gb[:], tok_emb.ap(), ids_sb[:], num_idxs=c.S,
                    num_idxs_reg=c.S, elem_size=c.D)

                s0 = embp.tile([128, c.D], F32)
                nc.sync.dma_start(s0[:], seg0rep.ap())
                sd = embp.tile([128, c.D], F32)
                nc.sync.dma_start(sd[:], segdrep.ap())
                ssel = embp.tile([128, c.TT], F32)
                nc.sync.dma_start(ssel[:], segsel.ap())
                mt = embp.tile([128, c.TT], F32)
                nc.sync.dma_start(mt[:], maskt.ap())
                eg = embp.tile([128, c.D], F32)
                nc.sync.dma_start(eg[:], egrep.ap())
                eb = embp.tile([128, c.D], F32)
                nc.sync.dma_start(eb[:], ebrep.ap())

                s0b = s0[:, None, :].to_broadcast((128, c.TT, c.D))
                nc.vector.tensor_tensor(gb[:], gb[:], s0b, OP.add)
                for tt in range(c.TT):
                    nc.vector.scalar_tensor_tensor(
                        gb[:, tt], sd[:], ssel[:, tt:tt + 1], gb[:, tt],
                        OP.mult, OP.add)

                mean = embp.tile([128, c.TT, 1], F32)
                nc.vector.tensor_reduce(mean[:], gb[:], mybir.AxisListType.X, OP.add)
                nc.vector.tensor_scalar_mul(mean[:], mean[:], 1.0 / c.D)
                nc.vector.tensor_tensor(
                    gb[:], gb[:], mean[:].to_broadcast((128, c.TT, c.D)), OP.subtract)
                sq = embp.tile([128, c.TT, c.D], F32)
                nc.scalar.square(sq[:], gb[:])
                var = embp.tile([128, c.TT, 1], F32)
                nc.vector.tensor_reduce(var[:], sq[:], mybir.AxisListType.X, OP.add)
                nc.vector.tensor_scalar(
                    var[:], var[:], 1.0 / c.D, 1e-12, OP.mult, OP.add)
                rstd = embp.tile([128, c.TT, 1], F32)
                nc.vector.reciprocal(rstd[:], var[:])
                nc.scalar.sqrt(rstd[:], rstd[:])
                nc.vector.tensor_tensor(
                    gb[:], gb[:], rstd[:].to_broadcast((128, c.TT, c.D)), OP.mult)
                egb = eg[:, None, :].to_broadcast((128, c.TT, c.D))
                nc.vector.tensor_tensor(gb[:], gb[:], egb, OP.mult)
                ebb = eb[:, None, :].to_broadcast((128, c.TT, c.D))
                nc.vector.tensor_tensor(gb[:], gb[:], ebb, OP.add)
                for tt in range(c.TT):
                    nc.vector.tensor_scalar_mul(gb[:, tt], gb[:, tt], mt[:, tt:tt + 1])

                for tt in range(c.TT):
                    for dt in range(c.DT):
                        pst = embps.tile([128, 128], F32, tag="tp")
                        nc.tensor.transpose(
                            pst[:], gb[:, tt, dt * 128:(dt + 1) * 128], identT[:])
                        nc.scalar.copy(hT32[:, dt, tt * 128:(tt + 1) * 128], pst[:])
                        nc.vector.tensor_copy(
                            hTbf[:, dt, tt * 128:(tt + 1) * 128], pst[:])

            # ---------------- layers ----------------
            pend = None
            for l in range(c.L):
                pend = layer(nc, tc, c, l, hT32, hTbf, dramp, consts, ins,
                             pairs, pend)

            # final deferred LN2 + output
            with (
                tc.tile_pool(name="flnp", bufs=2) as flnp,
                tc.tile_pool(name="flns", bufs=1) as flns,
                tc.tile_pool(name="flnps", bufs=1, space="PSUM") as flnps,
                tc.tile_pool(name="flnpb", bufs=2, space="PSUM") as flnpb,
            ):
                g_sb = flns.tile([128, c.DT], F32)
                nc.sync.dma_start(g_sb[:], ins["ln2g"].ap()[c.L - 1])
                b_sb = flns.tile([128, c.DT], F32)
                nc.sync.dma_start(b_sb[:], ins["ln2b"].ap()[c.L - 1])
                for ch in range(c.NCH):
                    _ln_chunk(nc, c, flnp, flns, flnps, flnpb, pend[ch],
                              hT32, hTbf, g_sb, b_sb, ch, consts)

            nc.sync.dma_start(out_hT.ap(), hT32[:])

    nc.compile()
    return nc


def _allreduce(nc, c, pairs, src, dst):
    if c.n_cores == 1 or c.no_cc:
        nc.sync.dma_start(dst[:], src[:])
    else:
        nc.gpsimd.collective_compute(
            "AllReduce", OP.add, replica_groups=pairs,
            ins=[src.opt()], outs=[dst.opt()])


def _ln_chunk(nc, c, lp, lps, pps, pbs, x_dr, hT32, hTbf, g_sb, b_sb, ch,
              consts):
    """Feature-major layernorm on one token chunk; x_dr: [128, DT, CH] fp16
    (AllReduce output).  Writes hT32 (f32) and hTbf (bf16) chunk columns."""
    CH, DT = c.CH, c.DT
    cs = slice(ch * CH, (ch + 1) * CH)
    ardt = F32 if c.ar_f32 else BF16
    onesc = consts["ones32"] if c.ar_f32 else consts["ones16"]
    eps2, invD_row, D_row = consts["eps2"], consts["invD_row"], consts["D_row"]

    xt = lp.tile([128, DT, CH], ardt, tag="lnx")
    nc.sync.dma_start(xt[:], x_dr[:])
    stats0 = pps.tile([1, CH], F32, tag="s0")
    stats1 = pps.tile([1, CH], F32, tag="s1")
    for dt in range(DT):
        x2t = lp.tile([128, CH], ardt, tag="lnx2")
        nc.scalar.square(x2t[:], xt[:, dt])
        nc.tensor.matmul(stats0[:], lhsT=onesc[:], rhs=xt[:, dt],
                         start=(dt == 0), stop=(dt == DT - 1))
        nc.tensor.matmul(stats1[:], lhsT=onesc[:], rhs=x2t[:],
                         start=(dt == 0), stop=(dt == DT - 1))
    s0r = lps.tile([1, CH], F32, tag="s0r")
    nc.scalar.copy(s0r[:], stats0[:])
    s1r = lps.tile([1, CH], F32, tag="s1r")
    nc.scalar.copy(s1r[:], stats1[:])
    u = lps.tile([1, CH], F32, tag="u")
    nc.vector.tensor_tensor(u[:], s0r[:], s0r[:], OP.mult)
    nc.vector.scalar_tensor_tensor(
        u[:], s1r[:], float(c.D), u[:], OP.mult, OP.subtract)
    nc.scalar.activation(u[:], u[:], AF.Sqrt, bias=eps2[:], scale=1.0)
    rp = lps.tile([1, CH], F32, tag="rp")
    nc.vector.reciprocal(rp[:], u[:])
    pm = pbs.tile([128, CH], F32, tag="bc")
    nc.tensor.matmul(pm[:], lhsT=invD_row[:], rhs=s0r[:], start=True, stop=True)
    mu_b = lps.tile([128, CH], F32, tag="mub")
    nc.scalar.copy(mu_b[:], pm[:])
    pr = pbs.tile([128, CH], F32, tag="bc")
    nc.tensor.matmul(pr[:], lhsT=D_row[:], rhs=rp[:], start=True, stop=True)
    rs_b = lps.tile([128, CH], F32, tag="rsb")
    nc.scalar.copy(rs_b[:], pr[:])

    for dt in range(DT):
        t = lp.tile([128, CH], F32, tag="lnt")
        nc.vector.tensor_tensor(t[:], xt[:, dt], mu_b[:], OP.subtract)
        nc.vector.tensor_tensor(t[:], t[:], rs_b[:], OP.mult)
        nc.vector.tensor_scalar(
            hT32[:, dt, cs], t[:], g_sb[:, dt:dt + 1], b_sb[:, dt:dt + 1],
            OP.mult, OP.add)
        nc.scalar.copy(hTbf[:, dt, cs], hT32[:, dt, cs])


def layer(nc, tc, c, l, hT32, hTbf, dramp, consts, ins, pairs, pend):
    """Emit one layer.  `pend` = (aroc_ch0, aroc_ch1) fp16 DRAM tiles of the
    previous layer's pre-LN2 AllReduce output (deferred LN2), or None.
    Returns this layer's pend for the next."""
    S, D, CH, NCH = c.S, c.D, c.CH, c.NCH
    DT, TT, JT, FT = c.DT, c.TT, c.JT, c.FT
    identT, ident8 = consts["identT"], consts["ident8"]
    ones1x64, onesrow, mb_sb = consts["ones1x64"], consts["onesrow"], consts["mb_sb"]

    with (
        tc.tile_pool(name=f"l{l}_misc", bufs=1) as miscp,
        tc.tile_pool(name=f"l{l}_ctx", bufs=1) as ctxp,
    ):
        ctxT = ctxp.tile([128, JT, S], BF16, name="ctxT")
        bq_sb = miscp.tile([128, 2 * JT], F32, name="bq_sb")
        nc.sync.dma_start(bq_sb[:], ins["bqkv"].ap()[l])
        bv_sb = miscp.tile([128, c.DCL], F32, name="bv_sb")
        nc.sync.dma_start(bv_sb[:], ins["bvrep"].ap()[l])
        bo_sb = miscp.tile([1, D], BF16, name="bo_sb")
        nc.sync.dma_start(bo_sb[:], ins["bo2"].ap()[l])

        # ---- phase A: load weights/pos tables; qkv/v projections;
        #      interleaved deferred LN2 of the previous layer ----
        with (
            tc.tile_pool(name=f"l{l}_ab", bufs=1) as abp,
        ):
            qsT = abp.tile([128, JT, S], BF16, name="qsT")
            kT = abp.tile([128, JT, S], BF16, name="kT")
            v_sb = abp.tile([128, TT, c.NHL * 65], BF16, name="v_sb")
            poskr = abp.tile([128, JT, c.PW], BF16, name="poskr")
            posq = abp.tile([128, JT, c.PW], BF16, name="posq")

            with (
                tc.tile_pool(name=f"l{l}_wqk", bufs=1) as wqkp,
                tc.tile_pool(name=f"l{l}_pps", bufs=2, space="PSUM") as pps,
                tc.tile_pool(name=f"l{l}_alnp", bufs=2) as alnp,
                tc.tile_pool(name=f"l{l}_alns", bufs=1) as alns,
                tc.tile_pool(name=f"l{l}_alnps", bufs=1, space="PSUM") as alnps,
                tc.tile_pool(name=f"l{l}_alnpb", bufs=2, space="PSUM") as alnpb,
            ):
                wqkv_s = wqkp.tile([128, DT, 3 * c.DCL], BF16, name="wqkv_s")
                nc.sync.dma_start(wqkv_s[:], ins["wqkv"].ap()[l])

                if pend is not None:
                    pg_sb = alns.tile([128, DT], F32, name="pg_sb")
                    nc.sync.dma_start(pg_sb[:], ins["ln2g"].ap()[l - 1])
                    pb_sb = alns.tile([128, DT], F32, name="pb_sb")
                    nc.sync.dma_start(pb_sb[:], ins["ln2b"].ap()[l - 1])

                def qkv_chunk(ch):
                    for proj in range(2):  # 0=q, 1=k
                        dst = (qsT, kT)[proj]
                        scl = (c.scale, 1.0)[proj]
                        for jt in range(JT):
                            wof = proj * c.DCL + jt * 128
                            ps = pps.tile([128, CH], F32, tag="qkv")
                            mm_acc(nc, ps[:], wqkv_s[:, :, wof:wof + 128],
                                   hTbf[:, :, ch * CH:(ch + 1) * CH], DT,
                                   True, True)
                            nc.scalar.activation(
                                dst[:, jt, ch * CH:(ch + 1) * CH], ps[:],
                                AF.Identity,
                                bias=bq_sb[:, proj * JT + jt:proj * JT + jt + 1],
                                scale=scl)
                    ntt = CH // 128
                    for tt in range(ch * ntt, (ch + 1) * ntt):
                        ps = pps.tile([128, c.DCL], F32, tag="vproj")
                        mm_acc(nc, ps[:], hTbf[:, :, tt * 128:(tt + 1) * 128],
                               wqkv_s[:, :, 2 * c.DCL:3 * c.DCL], DT, True, True)
                        for hl in range(c.NHL):
                            nc.vector.tensor_tensor(
                                v_sb[:, tt, hl * 65:hl * 65 + 64],
                                ps[:, hl * 64:(hl + 1) * 64],
                                bv_sb[:, hl * 64:(hl + 1) * 64], OP.add)

                for ch in range(NCH):
                    if pend is not None:
                        _ln_chunk(nc, c, alnp, alns, alnps, alnpb, pend[ch],
                                  hT32, hTbf, pg_sb, pb_sb, ch, consts)
                    qkv_chunk(ch)
                # pos-table loads after the qkv GEMM emission so they don't
                # steal DMA bandwidth from the critical LN x reads at the
                # layer boundary (first use is phase B, plenty of slack)
                nc.sync.dma_start(poskr[:], ins["poskt"].ap()[l])
                nc.sync.dma_start(posq[:], ins["posqt"].ap()[l])
                for hl in range(c.NHL):
                    nc.vector.memset(v_sb[:, :, hl * 65 + 64:hl * 65 + 65], 1.0)

            # ---- phase B: per-head attention ----
            with (
                tc.tile_pool(name=f"l{l}_ct", bufs=4) as ctp,
                tc.tile_pool(name=f"l{l}_g1", bufs=3) as g1p,
                tc.tile_pool(name=f"l{l}_g2", bufs=3) as g2p,
                tc.tile_pool(name=f"l{l}_ex", bufs=2) as exp_,
                tc.tile_pool(name=f"l{l}_sc", bufs=2) as scp,
                tc.tile_pool(name=f"l{l}_bps", bufs=2, space="PSUM") as bps,
                tc.tile_pool(name=f"l{l}_bsc", bufs=3, space="PSUM") as bsc,
                tc.tile_pool(name=f"l{l}_bp2", bufs=2, space="PSUM") as bps2,
                tc.tile_pool(name=f"l{l}_bp3", bufs=1, space="PSUM") as bps3,
            ):
                for hl in range(c.NHL):
                    jt, rb = hl // 2, 64 * (hl % 2)
                    qh = qsT[rb:rb + 64, jt]      # [64, S]
                    kh = kT[rb:rb + 64, jt]
                    pkh = poskr[rb:rb + 64, jt]   # [64, PW]
                    pqh = posq[rb:rb + 64, jt]

                    W2 = c.W2                     # compact per-tile table width
                    cq_dr = dramp.tile([S, W2], FP8, tag="cq", name=f"cq{l}_{hl}")
                    ck_dr = dramp.tile([S, W2], FP8, tag="ck", name=f"ck{l}_{hl}")
                    # compact skew tables: tile rt's 128 rows only ever read a
                    # (W2-1)-wide sliding window, so each row-block is built
                    # against a shifted slice of the host pos table:
                    #   cq: T[r, c'] = q_s[q] * pos_k[clip(rt*128 + 639 - c')]
                    #       rhs slice offset 896 - rt*128 of PXr_k
                    #   ck: T[r, c'] = k[kt*128+r] * pos_q[clip(c'+385-kt*128)]
                    #       rhs slice offset 897 - kt*128 of PX_q
                    for which, (dr, lh, rh, ofs0) in enumerate(
                            ((cq_dr, qh, pkh, 896), (ck_dr, kh, pqh, 897))):
                        th = dr[:].tensor
                        base = dr[:].offset
                        for rt in range(TT):
                            st = ctp.tile([128, W2], FP8, tag="cstage")
                            off = ofs0 - rt * 128
                            for co in range(0, W2, 512):
                                w = min(512, W2 - co)
                                ps = bps.tile([128, 512], F32, tag="ctab")
                                nc.tensor.matmul(
                                    ps[:, :w], lhsT=lh[:, rt * 128:(rt + 1) * 128],
                                    rhs=rh[:, off + co:off + co + w],
                                    start=True, stop=True)
                                if which == 0:
                                    nc.scalar.activation(
                                        st[:, co:co + w], ps[:, :w],
                                        AF.Copy, scale=256.0)
                                else:
                                    nc.vector.tensor_scalar_mul(
                                        st[:, co:co + w], ps[:, :w], 256.0)
                            dst = bass.AP(
                                th, base + (rt * 128) * W2,
                                [[W2, 128], [1, W2]])
                            nc.sync.dma_start(dst, st[:])

                    g1 = g1p.tile([128, TT, S], FP8, tag="g1")
                    thq = cq_dr[:].tensor
                    bq_ = cq_dr[:].offset
                    for qt in range(TT):
                        src = bass.AP(thq, bq_ + W2 * (qt * 128) + 127,
                                      [[W2 - 1, 128], [1, S]])
                        nc.sync.dma_start(g1[:, qt], src)

                    ex = exp_.tile([128, TT, S], BF16, tag="ex")
                    thk = ck_dr[:].tensor
                    bk_ = ck_dr[:].offset
                    for kt in range(TT):
                        g2 = g2p.tile([128, S], FP8, tag="g2", name=f"g2_{kt}")
                        src = bass.AP(thk, bk_ + W2 * (kt * 128) + 127,
                                      [[W2 - 1, 128], [1, S]])
                        nc.sync.dma_start(g2[:], src)
                        for ch in range(NCH):
                            ps = bsc.tile([128, CH], F32, tag="scores")
                            nc.tensor.matmul(
                                ps[:], lhsT=kh[:, kt * 128:(kt + 1) * 128],
                                rhs=qh[:, ch * CH:(ch + 1) * CH],
                                start=True, stop=False)
                            nc.tensor.matmul(
                                ps[:], lhsT=ident8[:],
                                rhs=g2[:, ch * CH:(ch + 1) * CH],
                                start=False, stop=False)
                            nq = CH // 128
                            for qi in range(nq):
                                qt = ch * nq + qi
                                nc.tensor.matmul(
                                    ps[:, qi * 128:(qi + 1) * 128],
                                    lhsT=g1[:, qt, kt * 128:(kt + 1) * 128],
                                    rhs=ident8[:],
                                    start=False, stop=True,
                                    skip_group_check=(qi != nq - 1))
                            nc.scalar.activation(
                                ex[:, kt, ch * CH:(ch + 1) * CH], ps[:], AF.Exp,
                                bias=mb_sb[:, kt:kt + 1], scale=1.0)

                    for ch in range(NCH):
                        pv = bps2.tile([65, CH], F32, tag="pv")
                        for kt in range(TT):
                            nc.tensor.matmul(
                                pv[:], lhsT=v_sb[:, kt, hl * 65:hl * 65 + 65],
                                rhs=ex[:, kt, ch * CH:(ch + 1) * CH],
                                start=(kt == 0), stop=(kt == TT - 1))
                        rec = scp.tile([1, CH], F32, tag="rec")
                        nc.vector.reciprocal(rec[:], pv[64:65, :])
                        pb = bps3.tile([64, CH], F32, tag="recb")
                        nc.tensor.matmul(pb[:], lhsT=ones1x64[:], rhs=rec[:],
                                         start=True, stop=True)
                        rb_sb = scp.tile([64, CH], F32, tag="recbs")
                        nc.scalar.copy(rb_sb[:], pb[:])
                        nc.vector.tensor_tensor(
                            ctxT[rb:rb + 64, jt, ch * CH:(ch + 1) * CH],
                            pv[0:64, :], rb_sb[:], OP.mult)

        # ---- phase C: Wo + chunked AR + (deferred-emission) LN1 ----
        ardt = F32 if c.ar_f32 else BF16
        if c.ar_chunked:
            ar1c, ar1oc = [], []
            for ch in range(NCH):
                ar1c.append(dramp.tile([128, DT, CH], ardt, tag="arc",
                                       name=f"ar1i_{l}_{ch}"))
                ar1oc.append(dramp.tile([128, DT, CH], ardt, tag="aroc",
                                        name=f"ar1o_{l}_{ch}"))
        else:
            ar1f = dramp.tile([128, DT, S], ardt, tag="arf", name=f"ar1i_{l}")
            ar1of = dramp.tile([128, DT, S], ardt, tag="arof", name=f"ar1o_{l}")
            ar1c = [ar1f[:, :, ch * CH:(ch + 1) * CH] for ch in range(NCH)]
            ar1oc = [ar1of[:, :, ch * CH:(ch + 1) * CH] for ch in range(NCH)]
        with (
            tc.tile_pool(name=f"l{l}_wops", bufs=2, space="PSUM") as wops,
            tc.tile_pool(name=f"l{l}_wost", bufs=3) as wost,
            tc.tile_pool(name=f"l{l}_wo", bufs=1) as wopool,
        ):
            wos = wopool.tile([128, JT, D], BF16, tag="wo")
            nc.sync.dma_start(wos[:], ins["wo"].ap()[l])

            for ch in range(NCH):
                for dt in range(DT):
                    ps = wops.tile([128, CH], F32, tag="wo")
                    mm_acc(nc, ps[:], wos[:, :, dt * 128:(dt + 1) * 128],
                           ctxT[:, :, ch * CH:(ch + 1) * CH], JT, True, False)
                    nc.tensor.matmul(
                        ps[:], lhsT=bo_sb[:, dt * 128:(dt + 1) * 128],
                        rhs=onesrow[:], start=False, stop=True)
                    st = wost.tile([128, CH], ardt, tag="wost")
                    nc.vector.scalar_tensor_tensor(
                        st[:], hT32[:, dt, ch * CH:(ch + 1) * CH], 0.5,
                        ps[:], OP.mult, OP.add)
                    nc.sync.dma_start(ar1c[ch][:, dt], st[:])
                if c.ar_chunked:
                    _allreduce(nc, c, pairs, ar1c[ch], ar1oc[ch])
            if not c.ar_chunked:
                _allreduce(nc, c, pairs, ar1f, ar1of)

    # ---- phase D: FFN (chunk-pipelined) + chunked AR2 ----
    # (ctx/attention pools are closed; FFN weights + LN tiles fit now)
    with (
        tc.tile_pool(name=f"l{l}_dmisc", bufs=1) as dmiscp,
        tc.tile_pool(name=f"l{l}_clnp", bufs=2) as clnp,
        tc.tile_pool(name=f"l{l}_clns", bufs=1) as clns,
        tc.tile_pool(name=f"l{l}_clnps", bufs=1, space="PSUM") as clnps,
        tc.tile_pool(name=f"l{l}_clnpb", bufs=2, space="PSUM") as clnpb,
        tc.tile_pool(name=f"l{l}_gt", bufs=2) as gtp,
        tc.tile_pool(name=f"l{l}_w1", bufs=4) as w1pool,
        tc.tile_pool(name=f"l{l}_w2", bufs=1) as w2pool,
        tc.tile_pool(name=f"l{l}_f1ps", bufs=2, space="PSUM") as f1ps,
        tc.tile_pool(name=f"l{l}_f2ps", bufs=2, space="PSUM") as f2ps,
        tc.tile_pool(name=f"l{l}_fst", bufs=3) as fst,
    ):
        b1_sb2 = dmiscp.tile([128, FT], F32, name="b1_sb2")
        nc.sync.dma_start(b1_sb2[:], ins["b1"].ap()[l])
        b2_sb2 = dmiscp.tile([1, D], BF16, name="b2_sb2")
        nc.sync.dma_start(b2_sb2[:], ins["b22"].ap()[l])
        g1_sb = clns.tile([128, DT], F32, name="g1_sb")
        nc.sync.dma_start(g1_sb[:], ins["ln1g"].ap()[l])
        bn1_sb = clns.tile([128, DT], F32, name="bn1_sb")
        nc.sync.dma_start(bn1_sb[:], ins["ln1b"].ap()[l])
        w2s = w2pool.tile([128, FT, D], BF16, name="w2s")
        nc.sync.dma_start(w2s[:], ins["w2"].ap()[l])
        ardt = F32 if c.ar_f32 else BF16
        if c.ar_chunked:
            ar2c, ar2oc = [], []
            for ch in range(NCH):
                ar2c.append(dramp.tile([128, DT, CH], ardt, tag="arc",
                                       name=f"ar2i_{l}_{ch}"))
                ar2oc.append(dramp.tile([128, DT, CH], ardt, tag="aroc",
                                        name=f"ar2o_{l}_{ch}"))
        else:
            ar2f = dramp.tile([128, DT, S], ardt, tag="arf", name=f"ar2i_{l}")
            ar2of = dramp.tile([128, DT, S], ardt, tag="arof", name=f"ar2o_{l}")
            ar2c = [ar2f[:, :, ch * CH:(ch + 1) * CH] for ch in range(NCH)]
            ar2oc = [ar2of[:, :, ch * CH:(ch + 1) * CH] for ch in range(NCH)]

        gts = []

        def ffn1_chunk(ch):
            gt = gtp.tile([128, FT, CH], BF16, tag="gt", name=f"gt{ch}")
            gts.append(gt)
            for ft in range(FT):
                wt = w1pool.tile([128, DT, 128], BF16, tag="w1t")
                nc.sync.dma_start(
                    wt[:], ins["w1"].ap()[l, :, :, ft * 128:(ft + 1) * 128])
                ps = f1ps.tile([128, CH], F32, tag="f1")
                mm_acc(nc, ps[:], wt[:],
                       hTbf[:, :, ch * CH:(ch + 1) * CH], DT, True, True)
                nc.scalar.activation(
                    gt[:, ft], ps[:],
                    AF.Gelu if c.act == "gelu" else AF.Relu,
                    bias=b1_sb2[:, ft:ft + 1], scale=1.0)

        def ffn2_chunk(ch):
            for dt in range(DT):
                ps = f2ps.tile([128, CH], F32, tag="f2")
                mm_acc(nc, ps[:], w2s[:, :, dt * 128:(dt + 1) * 128],
                       gts[ch][:], FT, True, False)
                nc.tensor.matmul(
                    ps[:], lhsT=b2_sb2[:, dt * 128:(dt + 1) * 128],
                    rhs=onesrow[:], start=False, stop=True)
                st = fst.tile([128, CH], ardt, tag="fst")
                nc.vector.scalar_tensor_tensor(
                    st[:], hT32[:, dt, ch * CH:(ch + 1) * CH], 0.5,
                    ps[:], OP.mult, OP.add)
                nc.sync.dma_start(ar2c[ch][:, dt], st[:])
            if c.ar_chunked:
                _allreduce(nc, c, pairs, ar2c[ch], ar2oc[ch])

        # emission order pipelines AR1/AR2 behind the other chunk's
        # GEMMs: ln1(0), ffn1(0), ln1(1), ffn1(1), ffn2(0), AR2(0),
        # ffn2(1), AR2(1); LN2 is deferred to the next layer.
        _ln_chunk(nc, c, clnp, clns, clnps, clnpb, ar1oc[0],
                  hT32, hTbf, g1_sb, bn1_sb, 0, consts)
        ffn1_chunk(0)
        _ln_chunk(nc, c, clnp, clns, clnps, clnpb, ar1oc[1],
                  hT32, hTbf, g1_sb, bn1_sb, 1, consts)
        ffn1_chunk(1)
        ffn2_chunk(0)
        ffn2_chunk(1)
        if not c.ar_chunked:
            _allreduce(nc, c, pairs, ar2f, ar2of)

    return ar2oc


# ---------------------------------------------------------------------------
# host side
# ---------------------------------------------------------------------------

def host_prep(c, inputs):
    """Build per-core in_maps from full inputs."""
    bf = ml_dtypes.bfloat16
    f32 = np.float32
    ii = {k: np.asarray(v) for k, v in inputs.items()}
    S, D, L = c.S, c.D, c.L

    def tokmaj(vec):  # [S] -> [128, TT]   t = tt*128 + p
        return np.ascontiguousarray(vec.reshape(c.TT, 128).T)

    rel = ii["rel_emb"].astype(f32)  # [2*SPAN, D]

    in_maps = []
    for core in range(c.n_cores):
        b, half = core // 2, core % 2
        colr = slice(half * c.DCL, (half + 1) * c.DCL)
        fcol = slice(half * c.FL, (half + 1) * c.FL)

        ids = ii["input_ids"][b].astype(np.int64)
        w = np.zeros((16, S // 16), np.int16)
        for i in range(S):
            w[i % 16, i // 16] = ids[i]
        ids16 = np.tile(w, (8, 1))

        seg = ii["segment_ids"][b].astype(f32)
        mask = ii["attention_mask"][b].astype(f32)

        wq = ii["Wq"][:, :, colr].astype(f32)
        wk = ii["Wk"][:, :, colr].astype(f32)
        wv = ii["Wv"][:, :, colr].astype(f32)
        wqkv = np.concatenate([wq, wk, wv], axis=2)  # [L, D, 3*DCL]
        wqkv = wqkv.reshape(L, c.DT, 128, 3 * c.DCL).transpose(0, 2, 1, 3)

        bq = ii["bq"][:, colr].astype(f32) * c.scale
        bk = ii["bk"][:, colr].astype(f32)
        bv = ii["bv"][:, colr].astype(f32)
        bqkv = np.concatenate(
            [bq.reshape(L, c.JT, 128).transpose(0, 2, 1),
             bk.reshape(L, c.JT, 128).transpose(0, 2, 1)], axis=2)
        bvrep = np.broadcast_to(bv[:, None, :], (L, 128, c.DCL))

        # host-precomputed pos projections, indexed for the compact skew
        # tables: PXr_k[v] = pos_k[clip(1535-v)], PX_q[w] = pos_q[clip(w-512)]
        poskt = np.zeros((L, 128, c.JT, c.PW), f32)
        posqt = np.zeros((L, 128, c.JT, c.PW), f32)
        idx_k = np.clip(1535 - np.arange(c.PW), 0, 2 * c.SPAN - 1)
        idx_q = np.clip(np.arange(c.PW) - 512, 0, 2 * c.SPAN - 1)
        for l in range(L):
            pk = rel @ wk[l] + ii["bk"][l, colr].astype(f32)   # [2*SPAN, DCL]
            pq = (rel @ wq[l] + ii["bq"][l, colr].astype(f32)) * c.scale
            for tab, idx, dst in ((pk, idx_k, poskt), (pq, idx_q, posqt)):
                ext = tab[idx]                                 # [PW, DCL]
                dst[l] = ext.T.reshape(c.JT, 128, c.PW).transpose(1, 0, 2)

        wo_ = ii["Wo"][:, colr, :].astype(f32)
        wo_ = wo_.reshape(L, c.JT, 128, D).transpose(0, 2, 1, 3)
        bo2 = (ii["bo"].astype(f32) / 2.0)[:, None, :]

        w1_ = ii["W1"][:, :, fcol].astype(f32)
        w1_ = w1_.reshape(L, c.DT, 128, c.FL).transpose(0, 2, 1, 3)
        b1_ = ii["b1"][:, fcol].astype(f32).reshape(L, c.FT, 128).transpose(0, 2, 1)
        w2_ = ii["W2"][:, fcol, :].astype(f32)
        w2_ = w2_.reshape(L, c.FT, 128, D).transpose(0, 2, 1, 3)
        b22 = (ii["b2"].astype(f32) / 2.0)[:, None, :]

        m = {
            "ids16": ids16,
            "tok_emb": ii["tok_emb"].astype(f32),
            "segsel": tokmaj(seg),
            "seg0rep": np.broadcast_to(
                ii["seg_emb"][0].astype(f32), (128, D)).copy(),
            "segdrep": np.broadcast_to(
                (ii["seg_emb"][1] - ii["seg_emb"][0]).astype(f32),
                (128, D)).copy(),
            "maskt": tokmaj(mask),
            "maskbias": tokmaj(NEG * (1.0 - mask)),
            "egrep": np.broadcast_to(
                ii["emb_ln_g"].astype(f32), (128, D)).copy(),
            "ebrep": np.broadcast_to(
                ii["emb_ln_b"].astype(f32), (128, D)).copy(),
            "poskt": poskt.astype(bf),
            "posqt": posqt.astype(bf),
            "wqkv": wqkv.astype(bf),
            "bqkv": np.ascontiguousarray(bqkv),
            "bvrep": np.ascontiguousarray(bvrep),
            "wo": wo_.astype(bf),
            "bo2": bo2.astype(bf),
            "w1": w1_.astype(bf),
            "b1": np.ascontiguousarray(b1_),
            "w2": w2_.astype(bf),
            "b22": b22.astype(bf),
            "ln1g": ii["ln1_g"].astype(f32).reshape(
                L, c.DT, 128).transpose(0, 2, 1),
            "ln1b": ii["ln1_b"].astype(f32).reshape(
                L, c.DT, 128).transpose(0, 2, 1),
            "ln2g": ii["ln2_g"].astype(f32).reshape(
                L, c.DT, 128).transpose(0, 2, 1),
            "ln2b": ii["ln2_b"].astype(f32).reshape(
                L, c.DT, 128).transpose(0, 2, 1),
        }
        m = {k: np.ascontiguousarray(v) for k, v in m.items()}
        in_maps.append(m)
    return in_maps


def assemble(c, results):
    """results[core]["out_hT"] [128, DT, S] -> [B, S, D] fp32."""
    out = np.zeros((c.B, c.S, c.D), np.float32)
    for b in range(c.B):
        hT = results[2 * b]["out_hT"]  # [128, DT, S]
        out[b] = hT.transpose(2, 1, 0).reshape(c.S, c.D)
    return out


_nc_cache = {}


def _get_nc(c):
    key = (c.B, c.S, c.D, c.H, c.F, c.L, c.V, c.SPAN, c.n_cores)
    if key not in _nc_cache:
        _nc_cache[key] = build_nc(c)
    return _nc_cache[key]


def kernel(**inputs):
    from concourse import bass_utils
    c = Cfg()
    nc = _get_nc(c)
    in_maps = host_prep(c, inputs)
    res = bass_utils.run_bass_kernel_spmd(
        nc, in_maps, core_ids=list(range(c.n_cores)))
    return assemble(c, res.results)


# revision 29
# speedup vs baseline: 1.0699x; 1.0699x over previous
"""DeBERTa-bare Trainium2 Bass kernel.

Topology: 8 NeuronCores = 4 data-parallel pairs (one batch element each) x
2-way tensor parallel (heads + FFN split) with pairwise AllReduce.

Everything on-chip runs feature-major ("transposed"): h is kept as
hT[d, token].  The DeBERTa disentangled-attention gathers
(take_along_axis over relative positions) are realized as affine "skew"
access-pattern DMA reads from DRAM-resident, clamp-extended c2p/p2c tables
(fp8, x256 scaled), injected into the score PSUM via scaled-identity
matmuls.

The pos_k/pos_q projections of the rel_embeddings table are precomputed on
the host (they only depend on weights).  The AllReduce/LayerNorm stream
runs in fp16, chunked over token halves so the collective overlaps the
other half's GEMMs; LN2 of each layer is deferred into the next layer's
QKV phase for the same reason.
"""

import sys

for _p in ("/opt/trn_rl_repo",):
    if _p not in sys.path:
        sys.path.insert(0, _p)

import numpy as np
import ml_dtypes

import concourse.bass as bass
import concourse.bacc as bacc
import concourse.tile as tile
import concourse.mybir as mybir
from concourse.masks import make_identity

F32 = mybir.dt.float32
BF16 = mybir.dt.bfloat16
FP16 = mybir.dt.float16
FP8 = mybir.dt.float8e4
I16 = mybir.dt.int16

AF = mybir.ActivationFunctionType
OP = mybir.AluOpType

NEG = -1e9


def mm_acc(nc, ps, lhsT3, rhs3, nsub, start, stop):
    """Accumulating matmul over `nsub` 128-contraction subtiles.
    lhsT3/rhs3: APs shaped [128, nsub, *]."""
    for s in range(nsub):
        nc.tensor.matmul(ps, lhsT3[:, s], rhs3[:, s],
                         start=(start and s == 0), stop=(stop and s == nsub - 1))


class Cfg:
    def __init__(self, B=4, S=1024, D=1024, H=16, F=4096, L=4, V=32000, SPAN=512,
                 n_cores=8, act="gelu", no_cc=False):
        self.B, self.S, self.D, self.H, self.F, self.L, self.V, self.SPAN = (
            B, S, D, H, F, L, V, SPAN)
        self.n_cores = n_cores
        self.DH = D // H
        assert self.DH == 64
        self.DT = D // 128          # d tiles
        self.TT = S // 128          # token tiles
        self.NHL = H // 2           # heads per core
        self.DCL = self.NHL * self.DH   # local head-dim cols
        self.JT = self.DCL // 128   # local dcol tiles (2 heads per tile)
        self.FL = F // 2            # local ffn cols
        self.FT = self.FL // 128
        self.CH = min(512, S)       # token chunk
        self.NCH = S // self.CH
        self.SUB = min(4, self.DT)
        self.W2 = S + 128           # compact per-tile skew table width
        self.PW = 2176              # host pos tables, clamp-extended
        self.scale = 1.0 / np.sqrt(3.0 * self.DH)
        self.act = act
        self.no_cc = no_cc
        self.ar_chunked = False     # one AR per LN point vs one per chunk
        self.ar_f32 = True          # AllReduce stream dtype (f32: CCE 16-bit path is slow)


def build_nc(cfg):
    c = cfg
    nc = bacc.Bacc("TRN2", target_bir_lowering=False, debug=False,
                   num_devices=c.n_cores)

    def inp(name, shape, dt):
        return nc.dram_tensor(name, list(shape), dt, kind="ExternalInput")

    ids16 = inp("ids16", [128, c.S // 16], I16)
    tok_emb = inp("tok_emb", [c.V, c.D], F32)
    segsel = inp("segsel", [128, c.TT], F32)
    seg0rep = inp("seg0rep", [128, c.D], F32)
    segdrep = inp("segdrep", [128, c.D], F32)
    maskt = inp("maskt", [128, c.TT], F32)
    maskbias = inp("maskbias", [128, c.TT], F32)
    egrep = inp("egrep", [128, c.D], F32)
    ebrep = inp("ebrep", [128, c.D], F32)
    poskt = inp("poskt", [c.L, 128, c.JT, c.PW], BF16)
    posqt = inp("posqt", [c.L, 128, c.JT, c.PW], BF16)
    wqkv = inp("wqkv", [c.L, 128, c.DT, 3 * c.DCL], BF16)
    bqkv = inp("bqkv", [c.L, 128, 2 * c.JT], F32)
    bvrep = inp("bvrep", [c.L, 128, c.DCL], F32)
    wo = inp("wo", [c.L, 128, c.JT, c.D], BF16)
    bo2 = inp("bo2", [c.L, 1, c.D], BF16)
    w1 = inp("w1", [c.L, 128, c.DT, c.FL], BF16)
    b1 = inp("b1", [c.L, 128, c.FT], F32)
    w2 = inp("w2", [c.L, 128, c.FT, c.D], BF16)
    b22 = inp("b22", [c.L, 1, c.D], BF16)
    ln1g = inp("ln1g", [c.L, 128, c.DT], F32)
    ln1b = inp("ln1b", [c.L, 128, c.DT], F32)
    ln2g = inp("ln2g", [c.L, 128, c.DT], F32)
    ln2b = inp("ln2b", [c.L, 128, c.DT], F32)

    out_hT = nc.dram_tensor("out_hT", [128, c.DT, c.S], F32, kind="ExternalOutput")

    pairs = [[2 * i, 2 * i + 1] for i in range(c.n_cores // 2)]

    with tile.TileContext(nc) as tc:
        import contextlib
        est = contextlib.ExitStack()
        with est:
            const = est.enter_context(tc.tile_pool(name="const", bufs=1))
            resid = est.enter_context(tc.tile_pool(name="resid", bufs=1))
            dramp = est.enter_context(tc.tile_pool(name="dramp", bufs=3, space="DRAM"))

            identT = const.tile([128, 128], F32)
            make_identity(nc, identT[:])
            ident8 = const.tile([128, 128], FP8)
            nc.gpsimd.memset(ident8[:], 2.0 ** -8)
            nc.gpsimd.affine_select(
                out=ident8[:], in_=ident8[:], compare_op=OP.is_equal, fill=0.0,
                base=0, pattern=[[-1, 128]], channel_multiplier=1)
            ones1x64 = const.tile([1, 64], F32)
            nc.vector.memset(ones1x64[:], 1.0)
            ones16 = const.tile([128, 1], BF16)
            nc.vector.memset(ones16[:], 1.0)
            ones32 = const.tile([128, 1], F32)
            nc.vector.memset(ones32[:], 1.0)
            onesrow = const.tile([1, c.CH], BF16)
            nc.vector.memset(onesrow[:], 1.0)
            eps2 = const.tile([1, 1], F32)
            nc.vector.memset(eps2[:], float(c.D) ** 2 * 1e-12)
            invD_row = const.tile([1, 128], F32)
            nc.vector.memset(invD_row[:], 1.0 / c.D)
            D_row = const.tile([1, 128], F32)
            nc.vector.memset(D_row[:], float(c.D))
            mb_sb = const.tile([128, c.TT], F32)
            nc.sync.dma_start(mb_sb[:], maskbias.ap())

            hT32 = resid.tile([128, c.DT, c.S], F32)
            hTbf = resid.tile([128, c.DT, c.S], BF16)

            consts = dict(identT=identT, ident8=ident8, ones1x64=ones1x64,
                          ones16=ones16, ones32=ones32, onesrow=onesrow,
                          eps2=eps2, invD_row=invD_row, D_row=D_row,
                          mb_sb=mb_sb)
            ins = dict(poskt=poskt, posqt=posqt, wqkv=wqkv, bqkv=bqkv,
                       bvrep=bvrep, wo=wo, bo2=bo2, w1=w1, b1=b1, w2=w2,
                       b22=b22, ln1g=ln1g, ln1b=ln1b, ln2g=ln2g, ln2b=ln2b)

            # ---------------- embedding ----------------
            with (
                tc.tile_pool(name="embp", bufs=1) as embp,
                tc.tile_pool(name="embps", bufs=2, space="PSUM") as embps,
            ):
                ids_sb = embp.tile([128, c.S // 16], I16)
                nc.sync.dma_start(ids_sb[:], ids16.ap())
                gb = embp.tile([128, c.TT, c.D], F32)
                nc.gpsimd.dma_gather(
                    gb[:], tok_emb.ap(), ids_sb[:], num_idxs=c.S,
                    num_idxs_reg=c.S, elem_size=c.D)

                s0 = embp.tile([128, c.D], F32)
                nc.sync.dma_start(s0[:], seg0rep.ap())
                sd = embp.tile([128, c.D], F32)
                nc.sync.dma_start(sd[:], segdrep.ap())
                ssel = embp.tile([128, c.TT], F32)
                nc.sync.dma_start(ssel[:], segsel.ap())
                mt = embp.tile([128, c.TT], F32)
                nc.sync.dma_start(mt[:], maskt.ap())
                eg = embp.tile([128, c.D], F32)
                nc.sync.dma_start(eg[:], egrep.ap())
                eb = embp.tile([128, c.D], F32)
                nc.sync.dma_start(eb[:], ebrep.ap())

                s0b = s0[:, None, :].to_broadcast((128, c.TT, c.D))
                nc.vector.tensor_tensor(gb[:], gb[:], s0b, OP.add)
                for tt in range(c.TT):
                    nc.vector.scalar_tensor_tensor(
                        gb[:, tt], sd[:], ssel[:, tt:tt + 1], gb[:, tt],
                        OP.mult, OP.add)

                mean = embp.tile([128, c.TT, 1], F32)
                nc.vector.tensor_reduce(mean[:], gb[:], mybir.AxisListType.X, OP.add)
                nc.vector.tensor_scalar_mul(mean[:], mean[:], 1.0 / c.D)
                nc.vector.tensor_tensor(
                    gb[:], gb[:], mean[:].to_broadcast((128, c.TT, c.D)), OP.subtract)
                sq = embp.tile([128, c.TT, c.D], F32)
                nc.scalar.square(sq[:], gb[:])
                var = embp.tile([128, c.TT, 1], F32)
                nc.vector.tensor_reduce(var[:], sq[:], mybir.AxisListType.X, OP.add)
                nc.vector.tensor_scalar(
                    var[:], var[:], 1.0 / c.D, 1e-12, OP.mult, OP.add)
                rstd = embp.tile([128, c.TT, 1], F32)
                nc.vector.reciprocal(rstd[:], var[:])
                nc.scalar.sqrt(rstd[:], rstd[:])
                nc.vector.tensor_tensor(
                    gb[:], gb[:], rstd[:].to_broadcast((128, c.TT, c.D)), OP.mult)
                egb = eg[:, None, :].to_broadcast((128, c.TT, c.D))
                nc.vector.tensor_tensor(gb[:], gb[:], egb, OP.mult)
                ebb = eb[:, None, :].to_broadcast((128, c.TT, c.D))
                nc.vector.tensor_tensor(gb[:], gb[:], ebb, OP.add)
                for tt in range(c.TT):
                    nc.vector.tensor_scalar_mul(gb[:, tt], gb[:, tt], mt[:, tt:tt + 1])

                for tt in range(c.TT):
                    for dt in range(c.DT):
                        pst = embps.tile([128, 128], F32, tag="tp")
                        nc.tensor.transpose(
                            pst[:], gb[:, tt, dt * 128:(dt + 1) * 128], identT[:])
                        nc.scalar.copy(hT32[:, dt, tt * 128:(tt + 1) * 128], pst[:])
                        nc.vector.tensor_copy(
                            hTbf[:, dt, tt * 128:(tt + 1) * 128], pst[:])

            # ---------------- layers ----------------
            pend = None
            for l in range(c.L):
                pend = layer(nc, tc, c, l, hT32, hTbf, dramp, consts, ins,
                             pairs, pend)

            # final deferred LN2 + output
            with (
                tc.tile_pool(name="flnp", bufs=2) as flnp,
                tc.tile_pool(name="flns", bufs=1) as flns,
                tc.tile_pool(name="flnps", bufs=1, space="PSUM") as flnps,
                tc.tile_pool(name="flnpb", bufs=2, space="PSUM") as flnpb,
            ):
                g_sb = flns.tile([128, c.DT], F32)
                nc.sync.dma_start(g_sb[:], ins["ln2g"].ap()[c.L - 1])
                b_sb = flns.tile([128, c.DT], F32)
                nc.sync.dma_start(b_sb[:], ins["ln2b"].ap()[c.L - 1])
                for ch in range(c.NCH):
                    _ln_chunk(nc, c, flnp, flns, flnps, flnpb, pend[ch],
                              hT32, hTbf, g_sb, b_sb, ch, consts)

            nc.sync.dma_start(out_hT.ap(), hT32[:])

    nc.compile()
    return nc


def _allreduce(nc, c, pairs, src, dst):
    if c.n_cores == 1 or c.no_cc:
        nc.sync.dma_start(dst[:], src[:])
    else:
        nc.gpsimd.collective_compute(
            "AllReduce", OP.add, replica_groups=pairs,
            ins=[src.opt()], outs=[dst.opt()])


def _ln_chunk(nc, c, lp, lps, pps, pbs, x_dr, hT32, hTbf, g_sb, b_sb, ch,
              consts):
    """Feature-major layernorm on one token chunk; x_dr: [128, DT, CH] fp16
    (AllReduce output).  Writes hT32 (f32) and hTbf (bf16) chunk columns."""
    CH, DT = c.CH, c.DT
    cs = slice(ch * CH, (ch + 1) * CH)
    ardt = F32 if c.ar_f32 else BF16
    onesc = consts["ones32"] if c.ar_f32 else consts["ones16"]
    eps2, invD_row, D_row = consts["eps2"], consts["invD_row"], consts["D_row"]

    xt = lp.tile([128, DT, CH], ardt, tag="lnx")
    nc.sync.dma_start(xt[:], x_dr[:])
    stats0 = pps.tile([1, CH], F32, tag="s0")
    stats1 = pps.tile([1, CH], F32, tag="s1")
    for dt in range(DT):
        x2t = lp.tile([128, CH], ardt, tag="lnx2")
        nc.scalar.square(x2t[:], xt[:, dt])
        nc.tensor.matmul(stats0[:], lhsT=onesc[:], rhs=xt[:, dt],
                         start=(dt == 0), stop=(dt == DT - 1))
        nc.tensor.matmul(stats1[:], lhsT=onesc[:], rhs=x2t[:],
                         start=(dt == 0), stop=(dt == DT - 1))
    s0r = lps.tile([1, CH], F32, tag="s0r")
    nc.scalar.copy(s0r[:], stats0[:])
    s1r = lps.tile([1, CH], F32, tag="s1r")
    nc.scalar.copy(s1r[:], stats1[:])
    u = lps.tile([1, CH], F32, tag="u")
    nc.vector.tensor_tensor(u[:], s0r[:], s0r[:], OP.mult)
    nc.vector.scalar_tensor_tensor(
        u[:], s1r[:], float(c.D), u[:], OP.mult, OP.subtract)
    nc.scalar.activation(u[:], u[:], AF.Sqrt, bias=eps2[:], scale=1.0)
    rp = lps.tile([1, CH], F32, tag="rp")
    nc.vector.reciprocal(rp[:], u[:])
    pm = pbs.tile([128, CH], F32, tag="bc")
    nc.tensor.matmul(pm[:], lhsT=invD_row[:], rhs=s0r[:], start=True, stop=True)
    mu_b = lps.tile([128, CH], F32, tag="mub")
    nc.scalar.copy(mu_b[:], pm[:])
    pr = pbs.tile([128, CH], F32, tag="bc")
    nc.tensor.matmul(pr[:], lhsT=D_row[:], rhs=rp[:], start=True, stop=True)
    rs_b = lps.tile([128, CH], F32, tag="rsb")
    nc.scalar.copy(rs_b[:], pr[:])

    for dt in range(DT):
        t = lp.tile([128, CH], F32, tag="lnt")
        nc.vector.tensor_tensor(t[:], xt[:, dt], mu_b[:], OP.subtract)
        nc.vector.tensor_tensor(t[:], t[:], rs_b[:], OP.mult)
        nc.vector.tensor_scalar(
            hT32[:, dt, cs], t[:], g_sb[:, dt:dt + 1], b_sb[:, dt:dt + 1],
            OP.mult, OP.add)
        nc.scalar.copy(hTbf[:, dt, cs], hT32[:, dt, cs])


def layer(nc, tc, c, l, hT32, hTbf, dramp, consts, ins, pairs, pend):
    """Emit one layer.  `pend` = (aroc_ch0, aroc_ch1) fp16 DRAM tiles of the
    previous layer's pre-LN2 AllReduce output (deferred LN2), or None.
    Returns this layer's pend for the next."""
    S, D, CH, NCH = c.S, c.D, c.CH, c.NCH
    DT, TT, JT, FT = c.DT, c.TT, c.JT, c.FT
    identT, ident8 = consts["identT"], consts["ident8"]
    ones1x64, onesrow, mb_sb = consts["ones1x64"], consts["onesrow"], consts["mb_sb"]

    with (
        tc.tile_pool(name=f"l{l}_misc", bufs=1) as miscp,
        tc.tile_pool(name=f"l{l}_ctx", bufs=1) as ctxp,
    ):
        ctxT = ctxp.tile([128, JT, S], BF16, name="ctxT")
        bq_sb = miscp.tile([128, 2 * JT], F32, name="bq_sb")
        nc.sync.dma_start(bq_sb[:], ins["bqkv"].ap()[l])
        bv_sb = miscp.tile([128, c.DCL], F32, name="bv_sb")
        nc.sync.dma_start(bv_sb[:], ins["bvrep"].ap()[l])
        bo_sb = miscp.tile([1, D], BF16, name="bo_sb")
        nc.sync.dma_start(bo_sb[:], ins["bo2"].ap()[l])

        # ---- phase A: load weights/pos tables; qkv/v projections;
        #      interleaved deferred LN2 of the previous layer ----
        with (
            tc.tile_pool(name=f"l{l}_ab", bufs=1) as abp,
        ):
            qsT = abp.tile([128, JT, S], BF16, name="qsT")
            kT = abp.tile([128, JT, S], BF16, name="kT")
            v_sb = abp.tile([128, TT, c.NHL * 65], BF16, name="v_sb")
            poskr = abp.tile([128, JT, c.PW], BF16, name="poskr")
            posq = abp.tile([128, JT, c.PW], BF16, name="posq")

            with (
                tc.tile_pool(name=f"l{l}_wqk", bufs=1) as wqkp,
                tc.tile_pool(name=f"l{l}_pps", bufs=2, space="PSUM") as pps,
                tc.tile_pool(name=f"l{l}_alnp", bufs=2) as alnp,
                tc.tile_pool(name=f"l{l}_alns", bufs=1) as alns,
                tc.tile_pool(name=f"l{l}_alnps", bufs=1, space="PSUM") as alnps,
                tc.tile_pool(name=f"l{l}_alnpb", bufs=2, space="PSUM") as alnpb,
            ):
                wqkv_s = wqkp.tile([128, DT, 3 * c.DCL], BF16, name="wqkv_s")
                nc.sync.dma_start(wqkv_s[:], ins["wqkv"].ap()[l])

                if pend is not None:
                    pg_sb = alns.tile([128, DT], F32, name="pg_sb")
                    nc.sync.dma_start(pg_sb[:], ins["ln2g"].ap()[l - 1])
                    pb_sb = alns.tile([128, DT], F32, name="pb_sb")
                    nc.sync.dma_start(pb_sb[:], ins["ln2b"].ap()[l - 1])

                def qkv_chunk(ch):
                    for proj in range(2):  # 0=q, 1=k
                        dst = (qsT, kT)[proj]
                        scl = (c.scale, 1.0)[proj]
                        for jt in range(JT):
                            wof = proj * c.DCL + jt * 128
                            ps = pps.tile([128, CH], F32, tag="qkv")
                            mm_acc(nc, ps[:], wqkv_s[:, :, wof:wof + 128],
                                   hTbf[:, :, ch * CH:(ch + 1) * CH], DT,
                                   True, True)
                            nc.scalar.activation(
                                dst[:, jt, ch * CH:(ch + 1) * CH], ps[:],
                                AF.Identity,
                                bias=bq_sb[:, proj * JT + jt:proj * JT + jt + 1],
                                scale=scl)
                    ntt = CH // 128
                    for tt in range(ch * ntt, (ch + 1) * ntt):
                        ps = pps.tile([128, c.DCL], F32, tag="vproj")
                        mm_acc(nc, ps[:], hTbf[:, :, tt * 128:(tt + 1) * 128],
                               wqkv_s[:, :, 2 * c.DCL:3 * c.DCL], DT, True, True)
                        for hl in range(c.NHL):
                            nc.vector.tensor_tensor(
                                v_sb[:, tt, hl * 65:hl * 65 + 64],
                                ps[:, hl * 64:(hl + 1) * 64],
                                bv_sb[:, hl * 64:(hl + 1) * 64], OP.add)

                for ch in range(NCH):
                    if pend is not None:
                        _ln_chunk(nc, c, alnp, alns, alnps, alnpb, pend[ch],
                                  hT32, hTbf, pg_sb, pb_sb, ch, consts)
                    qkv_chunk(ch)
                # pos-table loads after the qkv GEMM emission so they don't
                # steal DMA bandwidth from the critical LN x reads at the
                # layer boundary (first use is phase B, plenty of slack)
                nc.sync.dma_start(poskr[:], ins["poskt"].ap()[l])
                nc.sync.dma_start(posq[:], ins["posqt"].ap()[l])
                for hl in range(c.NHL):
                    nc.vector.memset(v_sb[:, :, hl * 65 + 64:hl * 65 + 65], 1.0)

            # ---- phase B: per-head attention ----
            with (
                tc.tile_pool(name=f"l{l}_ct", bufs=4) as ctp,
                tc.tile_pool(name=f"l{l}_g1", bufs=3) as g1p,
                tc.tile_pool(name=f"l{l}_g2", bufs=3) as g2p,
                tc.tile_pool(name=f"l{l}_ex", bufs=2) as exp_,
                tc.tile_pool(name=f"l{l}_sc", bufs=2) as scp,
                tc.tile_pool(name=f"l{l}_bps", bufs=2, space="PSUM") as bps,
                tc.tile_pool(name=f"l{l}_bsc", bufs=3, space="PSUM") as bsc,
                tc.tile_pool(name=f"l{l}_bp2", bufs=2, space="PSUM") as bps2,
                tc.tile_pool(name=f"l{l}_bp3", bufs=1, space="PSUM") as bps3,
            ):
                for hl in range(c.NHL):
                    jt, rb = hl // 2, 64 * (hl % 2)
                    qh = qsT[rb:rb + 64, jt]      # [64, S]
                    kh = kT[rb:rb + 64, jt]
                    pkh = poskr[rb:rb + 64, jt]   # [64, PW]
                    pqh = posq[rb:rb + 64, jt]

                    W2 = c.W2                     # compact per-tile table width
                    cq_dr = dramp.tile([S, W2], FP8, tag="cq", name=f"cq{l}_{hl}")
                    ck_dr = dramp.tile([S, W2], FP8, tag="ck", name=f"ck{l}_{hl}")
                    # compact skew tables: tile rt's 128 rows only ever read a
                    # (W2-1)-wide sliding window, so each row-block is built
                    # against a shifted slice of the host pos table:
                    #   cq: T[r, c'] = q_s[q] * pos_k[clip(rt*128 + 639 - c')]
                    #       rhs slice offset 896 - rt*128 of PXr_k
                    #   ck: T[r, c'] = k[kt*128+r] * pos_q[clip(c'+385-kt*128)]
                    #       rhs slice offset 897 - kt*128 of PX_q
                    for which, (dr, lh, rh, ofs0) in enumerate(
                            ((cq_dr, qh, pkh, 896), (ck_dr, kh, pqh, 897))):
                        th = dr[:].tensor
                        base = dr[:].offset
                        for rt in range(TT):
                            st = ctp.tile([128, W2], FP8, tag="cstage")
                            off = ofs0 - rt * 128
                            for co in range(0, W2, 512):
                                w = min(512, W2 - co)
                                ps = bps.tile([128, 512], F32, tag="ctab")
                                nc.tensor.matmul(
                                    ps[:, :w], lhsT=lh[:, rt * 128:(rt + 1) * 128],
                                    rhs=rh[:, off + co:off + co + w],
                                    start=True, stop=True)
                                if which == 0:
                                    nc.scalar.activation(
                                        st[:, co:co + w], ps[:, :w],
                                        AF.Copy, scale=256.0)
                                else:
                                    nc.vector.tensor_scalar_mul(
                                        st[:, co:co + w], ps[:, :w], 256.0)
                            dst = bass.AP(
                                th, base + (rt * 128) * W2,
                                [[W2, 128], [1, W2]])
                            nc.sync.dma_start(dst, st[:])

                    g1 = g1p.tile([128, TT, S], FP8, tag="g1")
                    thq = cq_dr[:].tensor
                    bq_ = cq_dr[:].offset
                    for qt in range(TT):
                        src = bass.AP(thq, bq_ + W2 * (qt * 128) + 127,
                                      [[W2 - 1, 128], [1, S]])
                        nc.sync.dma_start(g1[:, qt], src)

                    ex = exp_.tile([128, TT, S], BF16, tag="ex")
                    thk = ck_dr[:].tensor
                    bk_ = ck_dr[:].offset
                    for kt in range(TT):
                        g2 = g2p.tile([128, S], FP8, tag="g2", name=f"g2_{kt}")
                        src = bass.AP(thk, bk_ + W2 * (kt * 128) + 127,
                                      [[W2 - 1, 128], [1, S]])
                        nc.sync.dma_start(g2[:], src)
                        for ch in range(NCH):
                            ps = bsc.tile([128, CH], F32, tag="scores")
                            nc.tensor.matmul(
                                ps[:], lhsT=kh[:, kt * 128:(kt + 1) * 128],
                                rhs=qh[:, ch * CH:(ch + 1) * CH],
                                start=True, stop=False)
                            nc.tensor.matmul(
                                ps[:], lhsT=ident8[:],
                                rhs=g2[:, ch * CH:(ch + 1) * CH],
                                start=False, stop=False)
                            nq = CH // 128
                            for qi in range(nq):
                                qt = ch * nq + qi
                                nc.tensor.matmul(
                                    ps[:, qi * 128:(qi + 1) * 128],
                                    lhsT=g1[:, qt, kt * 128:(kt + 1) * 128],
                                    rhs=ident8[:],
                                    start=False, stop=True,
                                    skip_group_check=(qi != nq - 1))
                            nc.scalar.activation(
                                ex[:, kt, ch * CH:(ch + 1) * CH], ps[:], AF.Exp,
                                bias=mb_sb[:, kt:kt + 1], scale=1.0)

                    for ch in range(NCH):
                        pv = bps2.tile([65, CH], F32, tag="pv")
                        for kt in range(TT):
                            nc.tensor.matmul(
                                pv[:], lhsT=v_sb[:, kt, hl * 65:hl * 65 + 65],
                                rhs=ex[:, kt, ch * CH:(ch + 1) * CH],
                                start=(kt == 0), stop=(kt == TT - 1))
                        rec = scp.tile([1, CH], F32, tag="rec")
                        nc.vector.reciprocal(rec[:], pv[64:65, :])
                        pb = bps3.tile([64, CH], F32, tag="recb")
                        nc.tensor.matmul(pb[:], lhsT=ones1x64[:], rhs=rec[:],
                                         start=True, stop=True)
                        rb_sb = scp.tile([64, CH], F32, tag="recbs")
                        nc.scalar.copy(rb_sb[:], pb[:])
                        nc.vector.tensor_tensor(
                            ctxT[rb:rb + 64, jt, ch * CH:(ch + 1) * CH],
                            pv[0:64, :], rb_sb[:], OP.mult)

        # ---- phase C: Wo + chunked AR + (deferred-emission) LN1 ----
        ardt = F32 if c.ar_f32 else BF16
        if c.ar_chunked:
            ar1c, ar1oc = [], []
            for ch in range(NCH):
                ar1c.append(dramp.tile([128, DT, CH], ardt, tag="arc",
                                       name=f"ar1i_{l}_{ch}"))
                ar1oc.append(dramp.tile([128, DT, CH], ardt, tag="aroc",
                                        name=f"ar1o_{l}_{ch}"))
        else:
            ar1f = dramp.tile([128, DT, S], ardt, tag="arf", name=f"ar1i_{l}")
            ar1of = dramp.tile([128, DT, S], ardt, tag="arof", name=f"ar1o_{l}")
            ar1c = [ar1f[:, :, ch * CH:(ch + 1) * CH] for ch in range(NCH)]
            ar1oc = [ar1of[:, :, ch * CH:(ch + 1) * CH] for ch in range(NCH)]
        with (
            tc.tile_pool(name=f"l{l}_wops", bufs=3, space="PSUM") as wops,
            tc.tile_pool(name=f"l{l}_wost", bufs=3) as wost,
            tc.tile_pool(name=f"l{l}_wo", bufs=1) as wopool,
        ):
            wos = wopool.tile([128, JT, D], BF16, tag="wo")
            nc.sync.dma_start(wos[:], ins["wo"].ap()[l])

            for ch in range(NCH):
                for dt in range(DT):
                    ps = wops.tile([128, CH], F32, tag="wo")
                    mm_acc(nc, ps[:], wos[:, :, dt * 128:(dt + 1) * 128],
                           ctxT[:, :, ch * CH:(ch + 1) * CH], JT, True, False)
                    nc.tensor.matmul(
                        ps[:], lhsT=bo_sb[:, dt * 128:(dt + 1) * 128],
                        rhs=onesrow[:], start=False, stop=True)
                    st = wost.tile([128, CH], ardt, tag="wost")
                    nc.vector.scalar_tensor_tensor(
                        st[:], hT32[:, dt, ch * CH:(ch + 1) * CH], 0.5,
                        ps[:], OP.mult, OP.add)
                    nc.sync.dma_start(ar1c[ch][:, dt], st[:])
                if c.ar_chunked:
                    _allreduce(nc, c, pairs, ar1c[ch], ar1oc[ch])
            if not c.ar_chunked:
                _allreduce(nc, c, pairs, ar1f, ar1of)

    # ---- phase D: FFN (chunk-pipelined) + chunked AR2 ----
    # (ctx/attention pools are closed; FFN weights + LN tiles fit now)
    with (
        tc.tile_pool(name=f"l{l}_dmisc", bufs=1) as dmiscp,
        tc.tile_pool(name=f"l{l}_clnp", bufs=2) as clnp,
        tc.tile_pool(name=f"l{l}_clns", bufs=1) as clns,
        tc.tile_pool(name=f"l{l}_clnps", bufs=1, space="PSUM") as clnps,
        tc.tile_pool(name=f"l{l}_clnpb", bufs=2, space="PSUM") as clnpb,
        tc.tile_pool(name=f"l{l}_gt", bufs=2) as gtp,
        tc.tile_pool(name=f"l{l}_w1", bufs=4) as w1pool,
        tc.tile_pool(name=f"l{l}_w2", bufs=1) as w2pool,
        tc.tile_pool(name=f"l{l}_f1ps", bufs=2, space="PSUM") as f1ps,
        tc.tile_pool(name=f"l{l}_f2ps", bufs=2, space="PSUM") as f2ps,
        tc.tile_pool(name=f"l{l}_fst", bufs=3) as fst,
    ):
        b1_sb2 = dmiscp.tile([128, FT], F32, name="b1_sb2")
        nc.sync.dma_start(b1_sb2[:], ins["b1"].ap()[l])
        b2_sb2 = dmiscp.tile([1, D], BF16, name="b2_sb2")
        nc.sync.dma_start(b2_sb2[:], ins["b22"].ap()[l])
        g1_sb = clns.tile([128, DT], F32, name="g1_sb")
        nc.sync.dma_start(g1_sb[:], ins["ln1g"].ap()[l])
        bn1_sb = clns.tile([128, DT], F32, name="bn1_sb")
        nc.sync.dma_start(bn1_sb[:], ins["ln1b"].ap()[l])
        w2s = w2pool.tile([128, FT, D], BF16, name="w2s")
        nc.sync.dma_start(w2s[:], ins["w2"].ap()[l])
        ardt = F32 if c.ar_f32 else BF16
        if c.ar_chunked:
            ar2c, ar2oc = [], []
            for ch in range(NCH):
                ar2c.append(dramp.tile([128, DT, CH], ardt, tag="arc",
                                       name=f"ar2i_{l}_{ch}"))
                ar2oc.append(dramp.tile([128, DT, CH], ardt, tag="aroc",
                                        name=f"ar2o_{l}_{ch}"))
        else:
            ar2f = dramp.tile([128, DT, S], ardt, tag="arf", name=f"ar2i_{l}")
            ar2of = dramp.tile([128, DT, S], ardt, tag="arof", name=f"ar2o_{l}")
            ar2c = [ar2f[:, :, ch * CH:(ch + 1) * CH] for ch in range(NCH)]
            ar2oc = [ar2of[:, :, ch * CH:(ch + 1) * CH] for ch in range(NCH)]

        gts = []

        def ffn1_chunk(ch):
            gt = gtp.tile([128, FT, CH], BF16, tag="gt", name=f"gt{ch}")
            gts.append(gt)
            for ft in range(FT):
                wt = w1pool.tile([128, DT, 128], BF16, tag="w1t")
                nc.sync.dma_start(
                    wt[:], ins["w1"].ap()[l, :, :, ft * 128:(ft + 1) * 128])
                ps = f1ps.tile([128, CH], F32, tag="f1")
                mm_acc(nc, ps[:], wt[:],
                       hTbf[:, :, ch * CH:(ch + 1) * CH], DT, True, True)
                nc.scalar.activation(
                    gt[:, ft], ps[:],
                    AF.Gelu if c.act == "gelu" else AF.Relu,
                    bias=b1_sb2[:, ft:ft + 1], scale=1.0)

        def ffn2_chunk(ch):
            for dt in range(DT):
                ps = f2ps.tile([128, CH], F32, tag="f2")
                mm_acc(nc, ps[:], w2s[:, :, dt * 128:(dt + 1) * 128],
                       gts[ch][:], FT, True, False)
                nc.tensor.matmul(
                    ps[:], lhsT=b2_sb2[:, dt * 128:(dt + 1) * 128],
                    rhs=onesrow[:], start=False, stop=True)
                st = fst.tile([128, CH], ardt, tag="fst")
                nc.vector.scalar_tensor_tensor(
                    st[:], hT32[:, dt, ch * CH:(ch + 1) * CH], 0.5,
                    ps[:], OP.mult, OP.add)
                nc.sync.dma_start(ar2c[ch][:, dt], st[:])
            if c.ar_chunked:
                _allreduce(nc, c, pairs, ar2c[ch], ar2oc[ch])

        # emission order pipelines AR1/AR2 behind the other chunk's
        # GEMMs: ln1(0), ffn1(0), ln1(1), ffn1(1), ffn2(0), AR2(0),
        # ffn2(1), AR2(1); LN2 is deferred to the next layer.
        _ln_chunk(nc, c, clnp, clns, clnps, clnpb, ar1oc[0],
                  hT32, hTbf, g1_sb, bn1_sb, 0, consts)
        ffn1_chunk(0)
        _ln_chunk(nc, c, clnp, clns, clnps, clnpb, ar1oc[1],
                  hT32, hTbf, g1_sb, bn1_sb, 1, consts)
        ffn1_chunk(1)
        ffn2_chunk(0)
        ffn2_chunk(1)
        if not c.ar_chunked:
            _allreduce(nc, c, pairs, ar2f, ar2of)

    return ar2oc


# ---------------------------------------------------------------------------
# host side
# ---------------------------------------------------------------------------

def host_prep(c, inputs):
    """Build per-core in_maps from full inputs."""
    bf = ml_dtypes.bfloat16
    f32 = np.float32
    ii = {k: np.asarray(v) for k, v in inputs.items()}
    S, D, L = c.S, c.D, c.L

    def tokmaj(vec):  # [S] -> [128, TT]   t = tt*128 + p
        return np.ascontiguousarray(vec.reshape(c.TT, 128).T)

    rel = ii["rel_emb"].astype(f32)  # [2*SPAN, D]

    in_maps = []
    for core in range(c.n_cores):
        b, half = core // 2, core % 2
        colr = slice(half * c.DCL, (half + 1) * c.DCL)
        fcol = slice(half * c.FL, (half + 1) * c.FL)

        ids = ii["input_ids"][b].astype(np.int64)
        w = np.zeros((16, S // 16), np.int16)
        for i in range(S):
            w[i % 16, i // 16] = ids[i]
        ids16 = np.tile(w, (8, 1))

        seg = ii["segment_ids"][b].astype(f32)
        mask = ii["attention_mask"][b].astype(f32)

        wq = ii["Wq"][:, :, colr].astype(f32)
        wk = ii["Wk"][:, :, colr].astype(f32)
        wv = ii["Wv"][:, :, colr].astype(f32)
        wqkv = np.concatenate([wq, wk, wv], axis=2)  # [L, D, 3*DCL]
        wqkv = wqkv.reshape(L, c.DT, 128, 3 * c.DCL).transpose(0, 2, 1, 3)

        bq = ii["bq"][:, colr].astype(f32) * c.scale
        bk = ii["bk"][:, colr].astype(f32)
        bv = ii["bv"][:, colr].astype(f32)
        bqkv = np.concatenate(
            [bq.reshape(L, c.JT, 128).transpose(0, 2, 1),
             bk.reshape(L, c.JT, 128).transpose(0, 2, 1)], axis=2)
        bvrep = np.broadcast_to(bv[:, None, :], (L, 128, c.DCL))

        # host-precomputed pos projections, indexed for the compact skew
        # tables: PXr_k[v] = pos_k[clip(1535-v)], PX_q[w] = pos_q[clip(w-512)]
        poskt = np.zeros((L, 128, c.JT, c.PW), f32)
        posqt = np.zeros((L, 128, c.JT, c.PW), f32)
        idx_k = np.clip(1535 - np.arange(c.PW), 0, 2 * c.SPAN - 1)
        idx_q = np.clip(np.arange(c.PW) - 512, 0, 2 * c.SPAN - 1)
        for l in range(L):
            pk = rel @ wk[l] + ii["bk"][l, colr].astype(f32)   # [2*SPAN, DCL]
            pq = (rel @ wq[l] + ii["bq"][l, colr].astype(f32)) * c.scale
            for tab, idx, dst in ((pk, idx_k, poskt), (pq, idx_q, posqt)):
                ext = tab[idx]                                 # [PW, DCL]
                dst[l] = ext.T.reshape(c.JT, 128, c.PW).transpose(1, 0, 2)

        wo_ = ii["Wo"][:, colr, :].astype(f32)
        wo_ = wo_.reshape(L, c.JT, 128, D).transpose(0, 2, 1, 3)
        bo2 = (ii["bo"].astype(f32) / 2.0)[:, None, :]

        w1_ = ii["W1"][:, :, fcol].astype(f32)
        w1_ = w1_.reshape(L, c.DT, 128, c.FL).transpose(0, 2, 1, 3)
        b1_ = ii["b1"][:, fcol].astype(f32).reshape(L, c.FT, 128).transpose(0, 2, 1)
        w2_ = ii["W2"][:, fcol, :].astype(f32)
        w2_ = w2_.reshape(L, c.FT, 128, D).transpose(0, 2, 1, 3)
        b22 = (ii["b2"].astype(f32) / 2.0)[:, None, :]

        m = {
            "ids16": ids16,
            "tok_emb": ii["tok_emb"].astype(f32),
            "segsel": tokmaj(seg),
            "seg0rep": np.broadcast_to(
                ii["seg_emb"][0].astype(f32), (128, D)).copy(),
            "segdrep": np.broadcast_to(
                (ii["seg_emb"][1] - ii["seg_emb"][0]).astype(f32),
                (128, D)).copy(),
            "maskt": tokmaj(mask),
            "maskbias": tokmaj(NEG * (1.0 - mask)),
            "egrep": np.broadcast_to(
                ii["emb_ln_g"].astype(f32), (128, D)).copy(),
            "ebrep": np.broadcast_to(
                ii["emb_ln_b"].astype(f32), (128, D)).copy(),
            "poskt": poskt.astype(bf),
            "posqt": posqt.astype(bf),
            "wqkv": wqkv.astype(bf),
            "bqkv": np.ascontiguousarray(bqkv),
            "bvrep": np.ascontiguousarray(bvrep),
            "wo": wo_.astype(bf),
            "bo2": bo2.astype(bf),
            "w1": w1_.astype(bf),
            "b1": np.ascontiguousarray(b1_),
            "w2": w2_.astype(bf),
            "b22": b22.astype(bf),
            "ln1g": ii["ln1_g"].astype(f32).reshape(
                L, c.DT, 128).transpose(0, 2, 1),
            "ln1b": ii["ln1_b"].astype(f32).reshape(
                L, c.DT, 128).transpose(0, 2, 1),
            "ln2g": ii["ln2_g"].astype(f32).reshape(
                L, c.DT, 128).transpose(0, 2, 1),
            "ln2b": ii["ln2_b"].astype(f32).reshape(
                L, c.DT, 128).transpose(0, 2, 1),
        }
        m = {k: np.ascontiguousarray(v) for k, v in m.items()}
        in_maps.append(m)
    return in_maps


def assemble(c, results):
    """results[core]["out_hT"] [128, DT, S] -> [B, S, D] fp32."""
    out = np.zeros((c.B, c.S, c.D), np.float32)
    for b in range(c.B):
        hT = results[2 * b]["out_hT"]  # [128, DT, S]
        out[b] = hT.transpose(2, 1, 0).reshape(c.S, c.D)
    return out


_nc_cache = {}


def _get_nc(c):
    key = (c.B, c.S, c.D, c.H, c.F, c.L, c.V, c.SPAN, c.n_cores)
    if key not in _nc_cache:
        _nc_cache[key] = build_nc(c)
    return _nc_cache[key]


def kernel(**inputs):
    from concourse import bass_utils
    c = Cfg()
    nc = _get_nc(c)
    in_maps = host_prep(c, inputs)
    res = bass_utils.run_bass_kernel_spmd(
        nc, in_maps, core_ids=list(range(c.n_cores)))
    return assemble(c, res.results)
